# revision 1
# baseline (speedup 1.0000x reference)
"""Trainium2 Bass kernel for nn_DeepLinear (B=64, D=512, U=512).

Strategy
--------
Data-parallel over batch: each of the 8 NeuronCores handles 8 batch rows
with the full parameter set resident in SBUF (fp16).

Math (reference):
  xn  = LN(x)                       per-row over D
  l1  = lrelu(LN(xn*w1 + b1))       LN over (D,U,2) per batch elem
  l21 = sum_k l1*w21 + b21 ; l22 = sum_k l1*w22 + b22
  l2  = lrelu(LN(z2)), z2 = (l21,l22)
  l3  = sum_k l2*w3 + b3
  out = lrelu(sum_d (LN(l3) + xn) + bias)

Device-side simplifications used (validated by a structure check on the
actual inputs, with a numpy fallback for the general case):
  * b1=be1=b21=b22=be2=b3=0, g1>0, g2>0, g3 constant along d.
  * LN1 stats are closed-form in xn (t1 = xn*w1 is linear), computed on
    host: the device evaluates l1 = lrelu(w1*a1[b,d] - c1[b]) via one
    ScalarE Lrelu with per-partition scale/bias.
  * g1 is folded into w21/w22, g2 into w3 (host precompute).
  * Layer-3 LN + d-reduction collapse to S3[b,u] = sum_d l3 plus scalar
    stats, so the device only emits S3 and sum(l3^2); the final affine +
    lrelu runs on host.
"""

import numpy as np

B, D, U = 64, 512, 512
EPS = 1e-5
NCORES = 8
BLOC = B // NCORES      # 8 batch rows per core
NDT = D // 128          # 4 partition tiles of d
N2 = D * U * 2          # LN2 element count
N3 = D * U              # LN3 element count

_CACHE = {}

# Exposed for test.py introspection (the grading harness ignores it).
LAST_RESULTS = None


def _lrelu(t):
    return np.where(t >= 0, t, 0.01 * t)


def _structure_ok(i):
    g3 = i["g3"]
    return (
        np.all(i["b1"] == 0)
        and np.all(i["be1"] == 0)
        and np.all(i["g1"] > 0)
        and np.all(i["b21"] == 0)
        and np.all(i["b22"] == 0)
        and np.all(i["be2"] == 0)
        and np.all(i["g2"] > 0)
        and np.all(i["b3"] == 0)
        and np.all(g3 == g3[:1])
    )


def _reference_numpy(i):
    """General-case fallback (mirrors reference.py in numpy, fp32)."""

    def ln(t, g, b, axes):
        m = t.mean(axis=axes, keepdims=True)
        v = ((t - m) ** 2).mean(axis=axes, keepdims=True)
        return (t - m) / np.sqrt(v + EPS) * g + b

    x = i["x"].astype(np.float32)
    xn = ln(x, i["g0"], i["be0"], (-1,))[:, :, None, None]
    l1 = _lrelu(ln(xn * i["w1"] + i["b1"], i["g1"], i["be1"], (1, 2, 3)))
    l21 = np.sum(l1 * i["w21"], axis=-1, keepdims=True) + i["b21"]
    l22 = np.sum(l1 * i["w22"], axis=-1, keepdims=True) + i["b22"]
    z2 = np.concatenate((l21, l22), axis=-1)
    l2 = _lrelu(ln(z2, i["g2"], i["be2"], (1, 2, 3)))
    l3 = np.sum(l2 * i["w3"], axis=-1, keepdims=True) + i["b3"]
    out = ln(l3, i["g3"], i["be3"], (1, 2, 3)) + xn
    out = _lrelu(np.sum(out, axis=1) + i["bias"][:, None])
    return np.squeeze(out, axis=-1).astype(np.float32)


def _w_layout(a):
    """[D,U,2] fp -> device layout [128, 2*NDT, U] fp16 (k-major, d=dt*128+p)."""
    a = a.transpose(2, 0, 1)                    # [2, D, U]
    a = a.reshape(2, NDT, 128, U)               # [2, NDT, 128, U]
    a = a.transpose(2, 0, 1, 3)                 # [128, 2, NDT, U]
    return np.ascontiguousarray(a.reshape(128, 2 * NDT, U), dtype=np.float16)


def _lrelu_mul_op():
    """Custom DVE op: out = lrelu(in0*s0 + s1) * in1  (lrelu slope = imm2).

    Fuses the layer-2 LN affine + LeakyReLU + w3 multiply into one
    VectorE instruction, taking that work off the (bottleneck) ScalarE.
    """
    from concourse import dve_ops
    from concourse.dve_spec import (
        Spec, Src0, Src1, C0, C1, C2, lower, maxx, _has_src1 as has_src1,
    )
    from concourse.dve_uop import DveOpSpec

    name = "LRELU_AFF_MUL_ANT"
    if hasattr(dve_ops, name):
        return getattr(dve_ops, name)
    y = Src0 * C0 + C1
    spec = Spec(body=maxx(y, y * C2) * Src1)
    opcode = dve_ops._CUSTOM_DVE_ROW_BASE + len(dve_ops.OPS)
    shas = {}
    for ver in ("v3", "v4"):
        try:
            s = DveOpSpec(
                name=name, opcode=opcode, uops=lower(spec, ver=ver),
                rd1_en=has_src1(spec),
            )
            shas[ver] = s.sha(ver)
        except Exception:
            pass
    op = dve_ops.DveOp(name, spec, subdim=False, uops_sha=shas)
    dve_ops.OPS.append(op)
    dve_ops._SUB_OPCODE_FOR_NAME[name] = opcode
    dve_ops.CUSTOM_DVE_SPECS[name] = spec
    setattr(dve_ops, name, op)
    return op


def _build_bass():
    import concourse.bass as bass
    import concourse.bacc as bacc
    import concourse.tile as tile
    from concourse import mybir
    from contextlib import ExitStack

    lrelu_mul = _lrelu_mul_op()

    f16 = mybir.dt.float16
    f32 = mybir.dt.float32
    AF = mybir.ActivationFunctionType
    OP = mybir.AluOpType

    nc = bacc.Bacc("TRN2")

    w1h = nc.dram_tensor("w1h", [128, 2 * NDT, U], f16, kind="ExternalInput")
    w21h = nc.dram_tensor("w21h", [128, 2 * NDT, U], f16, kind="ExternalInput")
    w22h = nc.dram_tensor("w22h", [128, 2 * NDT, U], f16, kind="ExternalInput")
    w3h = nc.dram_tensor("w3h", [128, 2 * NDT, U], f16, kind="ExternalInput")
    # a1 (NDT*BLOC cols) and -c1 (BLOC cols) packed into one tensor/DMA so
    # downstream consumers wait on a single DMA queue semaphore.
    sch = nc.dram_tensor("sch", [128, (NDT + 1) * BLOC], f32, kind="ExternalInput")
    s3out = nc.dram_tensor("s3out", [BLOC, U], f32, kind="ExternalOutput")
    q3out = nc.dram_tensor("q3out", [128, BLOC], f32, kind="ExternalOutput")

    with ExitStack() as ctx:
        tc = ctx.enter_context(tile.TileContext(nc))
        wpool = ctx.enter_context(tc.tile_pool(name="wpool", bufs=1))
        zpool = ctx.enter_context(tc.tile_pool(name="zpool", bufs=1))
        lpool = ctx.enter_context(tc.tile_pool(name="lpool", bufs=3))
        ppool = ctx.enter_context(tc.tile_pool(name="ppool", bufs=4))
        jpool = ctx.enter_context(tc.tile_pool(name="jpool", bufs=2))
        spool = ctx.enter_context(tc.tile_pool(name="spool", bufs=1))
        pspool = ctx.enter_context(tc.tile_pool(name="pspool", bufs=1, space="PSUM"))
        dpool = ctx.enter_context(tc.tile_pool(name="dpool", bufs=1, space="DRAM"))

        # --- load weights + per-batch scalars -------------------------------
        # DMA queues are assigned round-robin in issue order and each queue
        # sustains only ~30 GB/s, so order by need-time and split the hot
        # tensors into per-dt chunks across queues: sch first (tiny, gates
        # everything), then w1 (gates lreluA), w21/w22 (gate the mults),
        # w3 last (not needed until phase B).
        schsb = spool.tile([128, (NDT + 1) * BLOC], f32)
        nc.sync.dma_start(out=schsb, in_=sch[:, :])
        w1sb = wpool.tile([128, 2 * NDT, U], f16)
        w21sb = wpool.tile([128, 2 * NDT, U], f16)
        w22sb = wpool.tile([128, 2 * NDT, U], f16)
        w3sb = wpool.tile([128, 2 * NDT, U], f16)
        for wsb, wh_ in ((w1sb, w1h), (w21sb, w21h), (w22sb, w22h)):
            hv = wh_[:, :, :].rearrange("p (k t) u -> p k t u", k=2)
            sv = wsb.rearrange("p (k t) u -> p k t u", k=2)
            for dt in range(NDT):
                nc.sync.dma_start(out=sv[:, :, dt, :], in_=hv[:, :, dt, :])
        nc.sync.dma_start(out=w3sb, in_=w3h[:, :, :])
        a1sb = schsb[:, 0 : NDT * BLOC].rearrange("p (t b) -> p t b", t=NDT)
        nc1sb = schsb[:, NDT * BLOC : (NDT + 1) * BLOC]

        # eye[p, b, j] = (b == j): per-b one-hot lhsT columns for the PE
        # row-selective colsum trick (built on-device; no extra DMA queue).
        eyesb = spool.tile([128, BLOC, BLOC], f16)
        nc.vector.memset(eyesb, 0.0)
        for b in range(BLOC):
            nc.vector.memset(eyesb[:, b, b : b + 1], 1.0)

        ones32 = spool.tile([128, 1], f32)
        nc.vector.memset(ones32, 1.0)
        zero128 = spool.tile([128, 1], f32)
        nc.vector.memset(zero128, 0.0)

        # z2 cache: col = b*(2*NDT) + k*NDT + dt  (each batch's slab contiguous)
        z2 = zpool.tile([128, 2 * BLOC * NDT, U], f16)
        statsQ2 = spool.tile([128, BLOC], f32)
        nc.vector.memset(statsQ2, 0.0)
        N_DVE_SQA = 2  # batches whose z2^2 runs on DVE+PE instead of ScalarE

        # LN2 stats split into two groups of 4 batches so group 0's
        # stats/broadcast chain overlaps phase A of batches 4-7.
        G0 = 3  # batches in stats group 0 (unblocks phase B earliest)
        GSZ = (G0, BLOC - G0)
        SApsA = pspool.tile([GSZ[0], U], f32)
        SApsB = pspool.tile([GSZ[1], U], f32)
        S3psum = pspool.tile([BLOC, U], f32)
        SQpA = pspool.tile([GSZ[0], 1], f32)
        SQpB = pspool.tile([GSZ[1], 1], f32)
        SQ2psum = pspool.tile([GSZ[0], U], f32)
        SAps = (SApsA, SApsB)
        SQp = (SQpA, SQpB)

        def grp(b):
            return (0, b, GSZ[0]) if b < G0 else (1, b - G0, GSZ[1])

        w1v = w1sb.rearrange("p (k t) u -> p k t u", k=2)

        # ============== per-group LN2 stats -> bcast chain ==================
        epsb = spool.tile([BLOC, 1], f32)
        nc.vector.memset(epsb, EPS)
        bcasts = [None, None]

        def emit_stats(g):
            gsz = GSZ[g]
            lo = 0 if g == 0 else G0
            SAr = spool.tile([gsz, 1], f32, name=f"SAr{g}")
            nc.vector.tensor_reduce(
                out=SAr, in_=SAps[g], axis=mybir.AxisListType.X, op=OP.add
            )
            nc.tensor.matmul(
                SQp[g], statsQ2[:, lo : lo + gsz], ones32,
                start=True, stop=True,
            )
            if g == 0:
                SQr2 = spool.tile([gsz, 1], f32, name="SQr2")
                nc.vector.tensor_reduce(
                    out=SQr2, in_=SQ2psum, axis=mybir.AxisListType.X, op=OP.add
                )
                SQsum = spool.tile([gsz, 1], f32, name="SQsum0")
                nc.vector.tensor_add(SQsum, SQp[g], SQr2)
            else:
                SQsum = SQp[g]
            pack = spool.tile([gsz, 2], f32, name=f"pack{g}")
            m2 = spool.tile([gsz, 1], f32, name=f"m2_{g}")
            nc.vector.tensor_scalar(
                out=m2, in0=SAr, scalar1=1.0 / N2, scalar2=None, op0=OP.mult
            )
            var2 = spool.tile([gsz, 1], f32, name=f"var2_{g}")
            nc.vector.tensor_scalar(
                out=var2, in0=SQsum, scalar1=1.0 / N2, scalar2=None, op0=OP.mult
            )
            msq = spool.tile([gsz, 1], f32, name=f"msq{g}")
            nc.vector.tensor_mul(msq, m2, m2)
            nc.vector.tensor_sub(var2, var2, msq)
            std2 = spool.tile([gsz, 1], f32, name=f"std2_{g}")
            nc.scalar.activation(out=std2, in_=var2, func=AF.Sqrt, bias=epsb[0:gsz])
            nc.vector.reciprocal(pack[:, 0:1], std2)
            nc.vector.tensor_mul(msq, m2, pack[:, 0:1])
            nc.vector.tensor_scalar(
                out=pack[:, 1:2], in0=msq, scalar1=-1.0, scalar2=None, op0=OP.mult
            )
            dscratch = dpool.tile([gsz, 2], f32, name=f"dscratch{g}")
            nc.sync.dma_start(out=dscratch, in_=pack)
            bc = spool.tile([128, gsz, 2], f32, name=f"bcast{g}")
            nc.sync.dma_start(
                out=bc,
                in_=bass.AP(
                    tensor=dscratch.tensor,
                    offset=dscratch.offset,
                    ap=[[0, 128]] + list(dscratch.ap),
                ),
            )
            bcasts[g] = bc

        # ============================ phase A ===============================
        def emit_sqA(b):
            junkA = ppool.tile([128, 2 * NDT, U], f16, tag="pp", name=f"junkA{b}")
            z2b = z2[:, b * 2 * NDT : (b + 1) * 2 * NDT, :]
            if b < N_DVE_SQA:
                # stock DVE square (2x mode) + PE colsums into SQ2psum row b
                nc.vector.tensor_mul(junkA, z2b, z2b)
                for c in range(2 * NDT):
                    nc.tensor.matmul(
                        SQ2psum,
                        eyesb[:, b, 0:G0],
                        junkA[:, c, :],
                        start=(b == 0 and c == 0),
                        stop=(b == N_DVE_SQA - 1 and c == 2 * NDT - 1),
                    )
            else:
                nc.scalar.activation(
                    out=junkA,
                    in_=z2b,
                    func=AF.Square,
                    bias=zero128,
                    accum_out=statsQ2[:, b : b + 1],
                )

        for b in range(BLOC):
            l1 = lpool.tile([128, 2 * NDT, U], f16, tag="l1")
            l1v = l1.rearrange("p (k t) u -> p k t u", k=2)
            for dt in range(NDT):
                nc.scalar.activation(
                    out=l1v[:, :, dt, :],
                    in_=w1v[:, :, dt, :],
                    func=AF.Lrelu,
                    bias=nc1sb[:, b : b + 1],
                    scale=a1sb[:, dt, b : b + 1],
                    alpha=0.01,
                )
            p21 = ppool.tile([128, 2 * NDT, U], f16, tag="pp")
            nc.vector.tensor_mul(p21, l1, w21sb)
            nc.vector.tensor_add(
                z2[:, b * 2 * NDT : b * 2 * NDT + NDT, :],
                p21[:, 0:NDT, :],
                p21[:, NDT : 2 * NDT, :],
            )
            p22 = ppool.tile([128, 2 * NDT, U], f16, tag="pp")
            nc.vector.tensor_mul(p22, l1, w22sb)
            nc.vector.tensor_add(
                z2[:, b * 2 * NDT + NDT : (b + 1) * 2 * NDT, :],
                p22[:, 0:NDT, :],
                p22[:, NDT : 2 * NDT, :],
            )
            # column sums of z2 into SApsum row b (sum over d and k).
            # lhsT = eyesb[:, b, :] is all-ones in column b, zero elsewhere,
            # so the [8,U] psum accumulates the colsum into row b only.
            g, r, gsz = grp(b)
            lo = 0 if g == 0 else G0
            for k in range(2):
                for dt in range(NDT):
                    nc.tensor.matmul(
                        SAps[g],
                        eyesb[:, b, lo : lo + gsz],
                        z2[:, b * 2 * NDT + k * NDT + dt, :],
                        start=(r == 0 and k == 0 and dt == 0),
                        stop=(r == gsz - 1 and k == 1 and dt == NDT - 1),
                    )
            if b > 0:
                emit_sqA(b - 1)
            if b == G0:
                emit_stats(0)
        emit_sqA(BLOC - 1)
        emit_stats(1)

        # ============================ phase B ===============================
        # For J_STOCK batch rows: ScalarE Lrelu + VectorE mul (keeps ScalarE
        # busy); for the rest: one fused custom VectorE op. Balances engines.
        J_STOCK = 3
        statsQ3 = spool.tile([128, BLOC], f32)
        for b in range(BLOC):
            z2b = z2[:, b * 2 * NDT : (b + 1) * 2 * NDT, :]
            p3 = ppool.tile([128, 2 * NDT, U], f16, tag="pp")
            if b < J_STOCK:
                l2 = lpool.tile([128, 2 * NDT, U], f16, tag="l2")
                nc.scalar.activation(
                    out=l2,
                    in_=z2b,
                    func=AF.Lrelu,
                    bias=bcasts[grp(b)[0]][:, grp(b)[1], 1:2],
                    scale=bcasts[grp(b)[0]][:, grp(b)[1], 0:1],
                    alpha=0.01,
                )
                nc.vector.tensor_mul(p3, l2, w3sb)
            else:
                nc.vector._custom_dve(
                    lrelu_mul,
                    out=p3.rearrange("p c u -> p (c u)"),
                    in0=z2b.rearrange("p c u -> p (c u)"),
                    in1=w3sb.rearrange("p c u -> p (c u)"),
                    s0=bcasts[grp(b)[0]][:, grp(b)[1], 0:1],
                    s1=bcasts[grp(b)[0]][:, grp(b)[1], 1:2],
                    imm2=0.01,
                )
            l3 = lpool.tile([128, NDT, U], f16, tag="l3")
            nc.vector.tensor_add(l3, p3[:, 0:NDT, :], p3[:, NDT : 2 * NDT, :])
            for dt in range(NDT):
                nc.tensor.matmul(
                    S3psum,
                    eyesb[:, b, :],
                    l3[:, dt, :],
                    start=(b == 0 and dt == 0),
                    stop=(b == BLOC - 1 and dt == NDT - 1),
                )
            junkB = jpool.tile([128, NDT, U], f16, tag="junkB", bufs=2)
            nc.scalar.activation(
                out=junkB,
                in_=l3,
                func=AF.Square,
                bias=zero128,
                accum_out=statsQ3[:, b : b + 1],
            )

        # ============================ outputs ===============================
        s3sb = spool.tile([BLOC, U], f32)
        nc.vector.tensor_copy(s3sb, S3psum)
        nc.sync.dma_start(out=s3out[:, :], in_=s3sb)
        nc.sync.dma_start(out=q3out[:, :], in_=statsQ3)

    nc.finalize()
    return nc


def _get_nc():
    if "nc" not in _CACHE:
        _CACHE["nc"] = _build_bass()
    return _CACHE["nc"]


def kernel(**inputs):
    global LAST_RESULTS
    i = {k: np.asarray(v) for k, v in inputs.items()}
    if not _structure_ok(i):
        return _reference_numpy(i)

    # If BASS_TRACE is set in the environment but the container's antenv stub
    # lacks axon_hooks, run_bass_kernel_spmd would crash on import; provide a
    # no-op hook module so tracing degrades gracefully instead.
    try:
        import antenv.axon_hooks  # noqa: F401
    except ImportError:
        import sys
        import types

        import antenv

        _m = types.ModuleType("antenv.axon_hooks")
        _h = {}
        _m.set_axon_ntff_profile_hook = lambda h: _h.__setitem__("hook", h)
        _m.get_axon_ntff_profile_hook = lambda: _h.get("hook")
        sys.modules["antenv.axon_hooks"] = _m
        antenv.axon_hooks = _m

    from concourse.bass_utils import run_bass_kernel_spmd

    # ---------------- host precompute (cheap, f64) -------------------------
    x = i["x"].astype(np.float64)
    g0 = i["g0"].astype(np.float64)
    be0 = i["be0"].astype(np.float64)
    mu = x.mean(axis=1, keepdims=True)
    v0 = ((x - mu) ** 2).mean(axis=1, keepdims=True)
    xn = (x - mu) / np.sqrt(v0 + EPS) * g0 + be0          # [B, D]

    w1 = i["w1"].astype(np.float64)[0]                    # [D, U, 2]
    g1 = i["g1"].astype(np.float64)
    wbar1 = w1.mean(axis=(1, 2))                          # [D]
    A1 = (w1 * w1).mean(axis=(1, 2))                      # [D]
    m1 = (xn @ wbar1) / D                                 # [B]
    E2 = ((xn * xn) @ A1) / D
    var1 = E2 - m1 * m1
    r1 = 1.0 / np.sqrt(var1 + EPS)                        # [B]
    a1 = xn * r1[:, None]                                 # [B, D]
    c1 = m1 * r1                                          # [B]
    X = xn.sum(axis=1)                                    # [B]

    w1dev = _w_layout(np.asarray(i["w1"][0], np.float32))
    w21dev = _w_layout((g1 * i["w21"][0]).astype(np.float32))
    w22dev = _w_layout((g1 * i["w22"][0]).astype(np.float32))
    w3dev = _w_layout((i["g2"].astype(np.float64) * i["w3"][0]).astype(np.float32))

    in_maps = []
    for c in range(NCORES):
        sl = slice(c * BLOC, (c + 1) * BLOC)
        a1c = a1[sl].astype(np.float32)                   # [BLOC, D]
        a1dev = a1c.reshape(BLOC, NDT, 128).transpose(2, 1, 0)  # [128, NDT, BLOC]
        nc1dev = np.broadcast_to(-c1[sl].astype(np.float32), (128, BLOC))
        schdev = np.concatenate(
            [a1dev.reshape(128, NDT * BLOC), nc1dev], axis=1
        ).astype(np.float32)
        in_maps.append(
            {
                "w1h": w1dev,
                "w21h": w21dev,
                "w22h": w22dev,
                "w3h": w3dev,
                "sch": np.ascontiguousarray(schdev),
            }
        )

    nc = _get_nc()
    res = run_bass_kernel_spmd(nc, in_maps, core_ids=list(range(NCORES)))
    LAST_RESULTS = res

    # ---------------- host finish ------------------------------------------
    S3 = np.concatenate(
        [res.results[c]["s3out"] for c in range(NCORES)], axis=0
    ).astype(np.float64)                                  # [B, U]
    q3 = np.concatenate(
        [res.results[c]["q3out"].sum(axis=0) for c in range(NCORES)], axis=0
    ).astype(np.float64)                                  # [B]

    m3 = S3.sum(axis=1) / N3
    var3 = q3 / N3 - m3 * m3
    r3 = 1.0 / np.sqrt(var3 + EPS)

    g3c = i["g3"].astype(np.float64)[0, :, 0]             # [U] (const along d)
    G3 = D * g3c
    Be3 = i["be3"].astype(np.float64)[:, :, 0].sum(axis=0)  # [U]
    bias = i["bias"].astype(np.float64)

    pre = (
        r3[:, None] * (g3c[None, :] * S3)
        - (m3 * r3)[:, None] * G3[None, :]
        + Be3[None, :]
        + X[:, None]
        + bias[None, :]
    )
    return _lrelu(pre).astype(np.float32)

